# revision 9
# baseline (speedup 1.0000x reference)
"""Trainium2 Bass kernel for CRFIntegrationModule.

Math: for each pixel the reference accumulates confidence-weighted depth
estimates from up to 16 same-semantic neighbors in 4 directions (L/R/U/D),
with multiplicative path weights exp(sum of log-gradients), then blends.

Reformulation (validated vs reference in numpy):
  With S = (msk==1 ? sem : -1), Q = (msk==1)*exp(-min(var,5)), q = Q*dep,
  b[n] = [S[n-1]==S[n]], the LEFT-direction accumulators satisfy segmented
  linear recurrences along the row:
    A[n] = b[n]*e^{g[n-1]}*(q[n-1] + A[n-1])      (depth sum, unwindowed)
    B[n] = b[n]*(Q[n-1] + B[n-1])                 (conf sum,  unwindowed)
  and the 16-neighbor window is recovered by subtracting the tail:
    W[n]  = A[n] - gate[n]*e^{C[n-1]-C[n-17]}*A[n-16]
    Wc[n] = B[n] - gate[n]*B[n-16]
  where C = cumsum(g), gate[n] = [NB[n]==NB[n-16]], NB = cumsum([S[n-1]!=S[n]]).
  These map to DVE tensor_tensor_scan ops. RIGHT uses the mirrored recurrence
  (negative-stride scan). U/D are identical along columns, computed in a
  PE-transposed layout (7 column chunks x 168 rows incl. 20-row halos).

Sharding: pure data parallel, 8 cores = 4 images x 2 row-halves (128 rows).
Each core loads its own 20 halo rows; no cross-device communication.
"""
import sys
import numpy as np

sys.path.insert(0, "/opt/trn_rl_repo")

BZ, H, W = 4, 256, 832
HB = 128              # rows per core
PAD = 20              # horizontal pad (window reads reach 17 back)
WP = W + 2 * PAD      # 872
HALO = 20             # vertical halo rows each side
HH = 64               # halo pack: top at partitions 0..19, bottom at 32..51
HB0, HB1 = 32, 52     # bottom-halo partition range (legal matmul base)
CH = HALO + HB + HALO  # 168 rows per transposed chunk
NCH = 7               # 832 = 6*128 + 64 column chunks
VW = NCH * CH         # 1176
CW_FULL = NCH * HB    # 896 (center width in transposed space)
CWS = [128] * 6 + [64]
CLIPVAR = 5.0
LAM = 0.05

_prog = None


def _build(stage=4):
    import concourse.tile as tile
    import concourse.mybir as mybir
    from concourse import bacc, masks
    from contextlib import ExitStack

    Alu = mybir.AluOpType
    Act = mybir.ActivationFunctionType
    f32 = mybir.dt.float32
    i32 = mybir.dt.int32

    nc = bacc.Bacc("TRN2", target_bir_lowering=False, debug=False)

    d_sem = nc.dram_tensor("sem", [HB, W], i32, kind="ExternalInput").ap()
    d_msk = nc.dram_tensor("msk", [HB, W], i32, kind="ExternalInput").ap()
    d_var = nc.dram_tensor("var", [HB, W], f32, kind="ExternalInput").ap()
    d_dep = nc.dram_tensor("dep", [HB, W], f32, kind="ExternalInput").ap()
    d_dpi = nc.dram_tensor("dpi", [HB, W], f32, kind="ExternalInput").ap()
    d_g0 = nc.dram_tensor("g0", [HB, W], f32, kind="ExternalInput").ap()
    d_g1 = nc.dram_tensor("g1", [HB, W], f32, kind="ExternalInput").ap()
    d_hsem = nc.dram_tensor("hsem", [HH, W], i32, kind="ExternalInput").ap()
    d_hmsk = nc.dram_tensor("hmsk", [HH, W], i32, kind="ExternalInput").ap()
    d_hvar = nc.dram_tensor("hvar", [HH, W], f32, kind="ExternalInput").ap()
    d_hdep = nc.dram_tensor("hdep", [HH, W], f32, kind="ExternalInput").ap()
    d_hg1 = nc.dram_tensor("hg1", [HH, W], f32, kind="ExternalInput").ap()
    d_out = nc.dram_tensor("out", [HB, W], f32, kind="ExternalOutput").ap()

    CS = slice(PAD, PAD + W)

    with tile.TileContext(nc) as tc, ExitStack() as ctx:
        pool = ctx.enter_context(tc.tile_pool(name="pool", bufs=1))
        psum = ctx.enter_context(tc.tile_pool(name="psum", bufs=1, space="PSUM"))

        # ---------- loads ----------
        semi = pool.tile([HB, W], i32)
        mski = pool.tile([HB, W], i32)
        var = pool.tile([HB, W], f32)
        dep = pool.tile([HB, W], f32)
        dpi = pool.tile([HB, W], f32)
        g1m = pool.tile([HB, W], f32)
        nc.sync.dma_start(semi[:], d_sem)
        nc.sync.dma_start(mski[:], d_msk)
        nc.sync.dma_start(var[:], d_var)
        nc.sync.dma_start(dep[:], d_dep)
        nc.sync.dma_start(dpi[:], d_dpi)
        nc.sync.dma_start(g1m[:], d_g1)
        g0h = pool.tile([HB, WP], f32)
        nc.gpsimd.memset(g0h[:], 0.0)
        nc.sync.dma_start(g0h[:, CS], d_g0)
        hsemi = pool.tile([HH, W], i32)
        hmski = pool.tile([HH, W], i32)
        hvar = pool.tile([HH, W], f32)
        hdep = pool.tile([HH, W], f32)
        hg1 = pool.tile([HH, W], f32)
        nc.sync.dma_start(hsemi[:], d_hsem)
        nc.sync.dma_start(hmski[:], d_hmsk)
        nc.sync.dma_start(hvar[:], d_hvar)
        nc.sync.dma_start(hdep[:], d_hdep)
        nc.sync.dma_start(hg1[:], d_hg1)

        # ---------- precompute S, Q, q (main + halo) ----------
        S = pool.tile([HB, WP], f32)
        Q = pool.tile([HB, WP], f32)
        q = pool.tile([HB, WP], f32)
        nc.gpsimd.memset(S[:], -1.0)
        nc.gpsimd.memset(Q[:], 0.0)
        nc.gpsimd.memset(q[:], 0.0)

        m1f = pool.tile([HB, W], f32)
        semf = pool.tile([HB, W], f32, tag="pre", bufs=2)
        nc.vector.tensor_copy(semf[:], semi[:])
        nc.vector.tensor_copy(m1f[:], mski[:])
        st = pool.tile([HB, W], f32, tag="pre", bufs=2)
        nc.vector.scalar_tensor_tensor(st[:], semf[:], 1.0, m1f[:],
                                       Alu.add, Alu.mult)
        nc.vector.tensor_scalar_sub(S[:, CS], st[:], 1.0)
        vm = pool.tile([HB, W], f32, tag="pre", bufs=2)
        nc.vector.tensor_scalar(vm[:], var[:], CLIPVAR, -1.0, Alu.min, Alu.mult)
        Etmp = pool.tile([HB, W], f32, tag="pre", bufs=2)
        nc.scalar.activation(Etmp[:], vm[:], Act.Exp)
        nc.vector.tensor_tensor(Q[:, CS], Etmp[:], m1f[:], Alu.mult)
        nc.vector.tensor_tensor(q[:, CS], Q[:, CS], dep[:], Alu.mult)

        hS = pool.tile([HH, W], f32)
        hQ = pool.tile([HH, W], f32)
        hq = pool.tile([HH, W], f32)
        hm1f = pool.tile([HH, W], f32)
        hsemf = pool.tile([HH, W], f32, tag="hpre", bufs=2)
        nc.vector.tensor_copy(hsemf[:], hsemi[:])
        nc.vector.tensor_copy(hm1f[:], hmski[:])
        hst = pool.tile([HH, W], f32, tag="hpre", bufs=2)
        nc.vector.scalar_tensor_tensor(hst[:], hsemf[:], 1.0, hm1f[:],
                                       Alu.add, Alu.mult)
        nc.vector.tensor_scalar_sub(hS[:], hst[:], 1.0)
        hvm = pool.tile([HH, W], f32, tag="hpre", bufs=2)
        nc.vector.tensor_scalar(hvm[:], hvar[:], CLIPVAR, -1.0, Alu.min, Alu.mult)
        hE = pool.tile([HH, W], f32, tag="hpre", bufs=2)
        nc.scalar.activation(hE[:], hvm[:], Act.Exp)
        nc.vector.tensor_tensor(hQ[:], hE[:], hm1f[:], Alu.mult)
        nc.vector.tensor_tensor(hq[:], hQ[:], hdep[:], Alu.mult)

        # ---------- transposed (vertical) planes ----------
        ident = pool.tile([HB, HB], f32)
        masks.make_identity(nc, ident[:])

        Sv = pool.tile([HB, VW], f32)
        Qv = pool.tile([HB, VW], f32)
        qv = pool.tile([HB, VW], f32)
        gv = pool.tile([HB, VW], f32)
        nc.gpsimd.memset(Sv[:], -1.0)
        nc.gpsimd.memset(Qv[:], 0.0)
        nc.gpsimd.memset(qv[:], 0.0)
        nc.gpsimd.memset(gv[:], 0.0)

        plane_srcs = [
            (S, hS, Sv, True),
            (Q, hQ, Qv, True),
            (q, hq, qv, True),
            (g1m, hg1, gv, False),
        ]
        for c in range(NCH if stage >= 2 else 0):
            cw = CWS[c]
            c0 = c * 128
            for (main, halo, dst, padded) in plane_srcs:
                mv = main[:, PAD + c0:PAD + c0 + cw] if padded else main[:, c0:c0 + cw]
                pt1 = psum.tile([HB, HB], f32, tag="ptin1", bufs=2)
                pt2 = psum.tile([HB, HALO], f32, tag="ptin2", bufs=2)
                pt3 = psum.tile([HB, HALO], f32, tag="ptin3", bufs=2)
                nc.tensor.transpose(pt1[:cw, :], mv, ident[:])
                nc.tensor.transpose(pt2[:cw, :], halo[0:HALO, c0:c0 + cw],
                                    ident[0:HALO, 0:HALO])
                nc.tensor.transpose(pt3[:cw, :], halo[HB0:HB1, c0:c0 + cw],
                                    ident[HB0:HB1, HB0:HB1])
                dslot = dst[:cw, c * CH:(c + 1) * CH]
                nc.scalar.copy(dslot[:, HALO:HALO + HB], pt1[:cw, :])
                nc.scalar.copy(dslot[:, 0:HALO], pt2[:cw, :])
                nc.scalar.copy(dslot[:, HALO + HB:CH], pt3[:cw, :])

        # ---------- direction passes ----------
        def directions(Sx, Qx, qx, gx, WD, dsum_add, csum_add):
            """Emit fwd+rev direction pair over [HB, WD] planes."""
            is_v = WD == VW
            CWIDTH = CW_FULL if is_v else W

            def c3(x):
                return x.rearrange("p (c f) -> p c f", c=NCH) if is_v else x

            def center(x, off):
                if not is_v:
                    return x[:, PAD + off:PAD + off + W]
                v = x.rearrange("p (c f) -> p c f", c=NCH)
                return v[:, :, HALO + off:HALO + off + HB]

            b = pool.tile([HB, VW], f32, tag="dir_b")
            nb = pool.tile([HB, VW], f32, tag="dir_be")
            nc.vector.tensor_tensor(b[:, 1:WD], Sx[:, :WD - 1], Sx[:, 1:WD],
                                    Alu.is_equal)
            nc.vector.tensor_tensor(nb[:, 1:WD], Sx[:, :WD - 1], Sx[:, 1:WD],
                                    Alu.not_equal)
            nc.vector.memset(b[:, 0:1], 0.0)
            nc.vector.memset(nb[:, 0:1], 1.0)
            if is_v:
                for c in range(1, NCH):
                    nc.vector.memset(b[:, c * CH:c * CH + 1], 0.0)
                    nc.vector.memset(nb[:, c * CH:c * CH + 1], 1.0)
            NBt = pool.tile([HB, VW], f32, tag="dir_NB")
            nc.vector.tensor_tensor_scan(NBt[:, :WD], nb[:, :WD], nb[:, :WD],
                                         0.0, Alu.add, Alu.bypass)
            Ct = pool.tile([HB, VW], f32, tag="dir_C")
            nc.vector.tensor_tensor_scan(Ct[:, :WD], gx[:, :WD], gx[:, :WD],
                                         0.0, Alu.add, Alu.bypass)
            Pt = pool.tile([HB, VW], f32, tag="dir_P")
            Pi = pool.tile([HB, VW], f32, tag="dir_Pi")
            nc.scalar.activation(Pt[:, :WD], Ct[:, :WD], Act.Exp)
            nc.scalar.activation(Pi[:, :WD], Ct[:, :WD], Act.Exp, scale=-1.0)

            for rev in (False, True):
                eg = pool.tile([HB, VW], f32, tag="dir_eg")
                be = pool.tile([HB, VW], f32, tag="dir_be")
                A = pool.tile([HB, VW], f32, tag="dir_A")
                B = pool.tile([HB, VW], f32, tag="dir_B")
                if not rev:
                    nc.scalar.activation(eg[:, :WD], gx[:, :WD], Act.Exp)
                    nc.vector.tensor_tensor(be[:, 1:WD], b[:, 1:WD],
                                            eg[:, :WD - 1], Alu.mult)
                    nc.vector.memset(A[:, 0:1], 0.0)
                    nc.vector.memset(B[:, 0:1], 0.0)
                    nc.vector.tensor_tensor_scan(
                        A[:, 1:WD], qx[:, :WD - 1], be[:, 1:WD],
                        0.0, Alu.add, Alu.mult)
                    nc.vector.tensor_tensor_scan(
                        B[:, 1:WD], Qx[:, :WD - 1], b[:, 1:WD],
                        0.0, Alu.add, Alu.mult)
                    g_o, t_o = 0, -16          # gate: NB[n] vs NB[n-16]
                    r_a, r_b = -1, -17         # ratio: P[n-1]*Pi[n-17]
                    RP, RPi = Pt, Pi
                else:
                    nc.scalar.activation(eg[:, :WD], gx[:, :WD], Act.Exp, scale=-1.0)
                    nc.vector.tensor_tensor(be[:, 0:WD - 1], b[:, 1:WD],
                                            eg[:, :WD - 1], Alu.mult)
                    nc.vector.memset(A[:, WD - 1:WD], 0.0)
                    nc.vector.memset(B[:, WD - 1:WD], 0.0)
                    nc.vector.tensor_tensor_scan(
                        A[:, 0:WD - 1][:, ::-1], qx[:, 1:WD][:, ::-1],
                        be[:, 0:WD - 1][:, ::-1], 0.0, Alu.add, Alu.mult)
                    nc.vector.tensor_tensor_scan(
                        B[:, 0:WD - 1][:, ::-1], Qx[:, 1:WD][:, ::-1],
                        b[:, 1:WD][:, ::-1], 0.0, Alu.add, Alu.mult)
                    g_o, t_o = 16, 16          # gate: NB[n+16] vs NB[n]
                    r_a, r_b = 15, -1          # ratio: Pi[n+15]*P[n-1]
                    RP, RPi = Pi, Pt

                gate = pool.tile([HB, CW_FULL], f32, tag="dir_gate")
                ratio = pool.tile([HB, CW_FULL], f32, tag="dir_ratio")
                TA = pool.tile([HB, CW_FULL], f32, tag="dir_TA")
                TB = pool.tile([HB, CW_FULL], f32, tag="dir_TB")
                gv_ = c3(gate[:, :CWIDTH])
                rv_ = c3(ratio[:, :CWIDTH])
                tv_ = c3(TA[:, :CWIDTH])
                tb_ = c3(TB[:, :CWIDTH])
                nc.vector.tensor_tensor(gv_, center(NBt, g_o), center(NBt, g_o - 16),
                                        Alu.is_equal)
                nc.vector.tensor_tensor(rv_, center(RP, r_a), center(RPi, r_b),
                                        Alu.mult)
                nc.vector.tensor_tensor(tv_, center(A, t_o), rv_, Alu.mult)
                nc.vector.tensor_tensor(tv_, tv_, gv_, Alu.mult)
                dsum_add('+', center(A, 0))
                dsum_add('-', tv_)
                nc.vector.tensor_tensor(tb_, center(B, t_o), gv_, Alu.mult)
                csum_add('+', center(B, 0))
                csum_add('-', tb_)

        # ---------- accumulators ----------
        dsum = pool.tile([HB, W], f32)
        csum = pool.tile([HB, W], f32)
        nc.scalar.copy(dsum[:], q[:, CS])
        nc.scalar.copy(csum[:], Q[:, CS])

        def h_acc(acc):
            def add(kind, ap):
                nc.vector.tensor_tensor(
                    acc[:], acc[:], ap, Alu.add if kind == '+' else Alu.subtract)
            return add

        if stage >= 1:
            directions(S, Q, q, g0h, WP, h_acc(dsum), h_acc(csum))

        Wv = pool.tile([HB, CW_FULL], f32)
        Wcv = pool.tile([HB, CW_FULL], f32)
        vstate = {"d": 0, "c": 0}

        def v_acc(acc, key):
            t = acc.rearrange("p (c f) -> p c f", c=NCH)

            def add(kind, ap):
                if vstate[key] == 0:
                    nc.vector.tensor_copy(t[:], ap)
                else:
                    nc.vector.tensor_tensor(
                        t[:], t[:], ap, Alu.add if kind == '+' else Alu.subtract)
                vstate[key] += 1
            return add

        if stage >= 3:
            directions(Sv, Qv, qv, gv, VW, v_acc(Wv, "d"), v_acc(Wcv, "c"))

        for c in range(NCH if stage >= 4 else 0):
            cw = CWS[c]
            c0 = c * 128
            for (src, acc) in ((Wv, dsum), (Wcv, csum)):
                pt = psum.tile([HB, HB], f32, tag="ptout", bufs=2)
                nc.tensor.transpose(pt[:HB, :cw], src[:cw, c * HB:c * HB + HB],
                                    ident[0:cw, 0:cw])
                nc.vector.tensor_tensor(acc[:, c0:c0 + cw], acc[:, c0:c0 + cw],
                                        pt[:HB, :cw], Alu.add)

        # ---------- final blend ----------
        mx = pool.tile([HB, W], f32, tag="fin", bufs=3)
        nc.vector.tensor_scalar_max(mx[:], csum[:], 1e-12)
        rcp = pool.tile([HB, W], f32, tag="fin", bufs=3)
        nc.vector.reciprocal(rcp[:], mx[:])
        lat = pool.tile([HB, W], f32, tag="fin", bufs=3)
        nc.vector.tensor_tensor(lat[:], dsum[:], rcp[:], Alu.mult)
        sel = pool.tile([HB, W], f32, tag="fin", bufs=3)
        nc.vector.tensor_scalar(sel[:], lat[:], 0.0, None, Alu.is_gt)
        dd = pool.tile([HB, W], f32, tag="fin", bufs=3)
        nc.vector.tensor_tensor(dd[:], lat[:], dpi[:], Alu.subtract)
        ee = pool.tile([HB, W], f32, tag="fin", bufs=3)
        nc.vector.tensor_tensor(ee[:], sel[:], dd[:], Alu.mult)
        outt = pool.tile([HB, W], f32, tag="fin", bufs=3)
        nc.vector.scalar_tensor_tensor(outt[:], ee[:], 1.0 - LAM, dpi[:],
                                       Alu.mult, Alu.add)
        nc.sync.dma_start(d_out, outt[:])

    nc.compile()
    return nc


def _get_prog():
    global _prog
    if _prog is None:
        _prog = _build()
    return _prog


def _core_maps(pred_log, semantics, mask, variance, dep_cur, dep_orig):
    maps = []
    for c in range(8):
        b, r = c // 2, c % 2
        r0 = r * HB
        sem = semantics[b, 0]
        msk = mask[b, 0]
        var = variance[b, 0]
        dep = dep_cur[b, 0]
        g1 = pred_log[b, 1]
        hsem = np.full((HH, W), -1, np.int32)
        hmsk = np.zeros((HH, W), np.int32)
        hvar = np.zeros((HH, W), np.float32)
        hdep = np.zeros((HH, W), np.float32)
        hg1 = np.zeros((HH, W), np.float32)
        if r0 - HALO >= 0:
            sl = slice(r0 - HALO, r0)
            hsem[:HALO] = sem[sl]
            hmsk[:HALO] = msk[sl]
            hvar[:HALO] = var[sl]
            hdep[:HALO] = dep[sl]
            hg1[:HALO] = g1[sl]
        if r0 + HB + HALO <= H:
            sl = slice(r0 + HB, r0 + HB + HALO)
            hsem[HB0:HB1] = sem[sl]
            hmsk[HB0:HB1] = msk[sl]
            hvar[HB0:HB1] = var[sl]
            hdep[HB0:HB1] = dep[sl]
            hg1[HB0:HB1] = g1[sl]
        rs = slice(r0, r0 + HB)
        maps.append({
            "sem": np.ascontiguousarray(sem[rs], np.int32),
            "msk": np.ascontiguousarray(msk[rs], np.int32),
            "var": np.ascontiguousarray(var[rs], np.float32),
            "dep": np.ascontiguousarray(dep[rs], np.float32),
            "dpi": np.ascontiguousarray(dep_orig[b, 0][rs], np.float32),
            "g0": np.ascontiguousarray(pred_log[b, 0][rs], np.float32),
            "g1": np.ascontiguousarray(g1[rs], np.float32),
            "hsem": hsem, "hmsk": hmsk, "hvar": hvar, "hdep": hdep, "hg1": hg1,
        })
    return maps


PROFILE = False
LAST_RESULT = None


def _run_once(pred_log, semantics, mask, variance, dep_cur, dep_orig):
    global LAST_RESULT
    from concourse.bass_utils import run_bass_kernel_spmd

    nc = _get_prog()
    in_maps = _core_maps(pred_log, semantics, mask, variance, dep_cur, dep_orig)
    res = run_bass_kernel_spmd(nc, in_maps, core_ids=list(range(8)),
                               trace=PROFILE)
    LAST_RESULT = res
    out = np.empty((BZ, 1, H, W), np.float32)
    for c in range(8):
        b, r = c // 2, c % 2
        out[b, 0, r * HB:(r + 1) * HB] = res.results[c]["out"]
    return out


def kernel(pred_log, semantics, mask, variance, depthin, times=1):
    pred_log = np.asarray(pred_log, np.float32)
    semantics = np.asarray(semantics)
    mask = np.asarray(mask)
    variance = np.asarray(variance, np.float32)
    depthin = np.asarray(depthin, np.float32).reshape(BZ, 1, H, W)
    t = int(np.asarray(times))
    depthout = depthin
    for _ in range(t):
        depthout = _run_once(pred_log, semantics, mask, variance,
                             depthout, depthin)
    if t == 0:
        depthout = depthin.copy()
    return depthout
